# revision 12
# baseline (speedup 1.0000x reference)
"""Multi-head self-attention (B=2, N=2048, C=1024, H=16) on 8 TRN2 NeuronCores.

Sharding: data-parallel over batch (2) x tensor-parallel over heads (16/4=4 groups).
Core c handles batch b=c//4 and heads [4*(c%4), 4*(c%4)+4).

Per-core kernel (all matmuls in float32r, 1 cycle/row at N>=256):
  1. QKV projection from x[b]^T (host passes the transpose; pure layout prep):
     Q^T,K^T computed as W^T @ X^T  -> [head-dim on partitions, seq free]
     V computed as X @ Wv           -> [seq on partitions, head-dim free] (natural)
  2. Attention per head: S^T = K^T.T @ Q^T (scores transposed), P^T = exp(S/8)
     on ACT, O_aug^T = [V|1]^T @ P^T accumulated over key tiles on PE; the
     appended ones-column yields the softmax denominators for free.
  3. Normalize: gather sums, DVE reciprocal, gpsimd partition_broadcast,
     DVE multiply; out-projection Y = O_norm @ W_out (i on partitions) with
     direct PSUM->DRAM stores.
Host sums the 4 per-batch partials (head groups) and adds biases.
"""

import numpy as np

import concourse.bass as bass
import concourse.bacc as bacc
import concourse.tile as tile
from concourse import library_config, mybir
from concourse.bass_utils import run_bass_kernel_spmd

B, NSEQ, CDIM, NHEADS, HD = 2, 2048, 1024, 16, 64
NH = 4          # heads per core
NCORES = 8
F32 = mybir.dt.float32
F32R = mybir.dt.float32r
EXP = mybir.ActivationFunctionType.Exp
SCALE = HD ** -0.5


def build_program():
    nc = bacc.Bacc("TRN2", target_bir_lowering=False, debug=False)

    xT = nc.dram_tensor("xT", [CDIM, NSEQ], F32R, kind="ExternalInput").ap()
    wqkv = nc.dram_tensor("wqkv", [CDIM, 3 * NH * HD], F32R, kind="ExternalInput").ap()
    wout = nc.dram_tensor("wout", [NH * HD, CDIM], F32R, kind="ExternalInput").ap()
    y = nc.dram_tensor("y", [NSEQ, CDIM], F32, kind="ExternalOutput").ap()
    dbg = nc.dram_tensor("dbg", [128, 8, 1024], F32, kind="ExternalOutput").ap()

    with tile.TileContext(nc) as tc:
        emit(nc, tc, xT, wqkv, wout, y, dbg)

    nc.compile()
    return nc


def emit(nc, tc, xT, wqkv, wout, y, dbg=None):
    import contextlib
    ctx = contextlib.ExitStack()
    with ctx:
        const = ctx.enter_context(tc.tile_pool(name="const", bufs=1))

        # ---- persistent SBUF tensors ----
        wqkv_sb = const.tile([128, 8, 3 * NH * HD], F32R)   # [p, ctile, 768]
        wout_sb = const.tile([128, 2, CDIM], F32R)          # [p, ktile, 1024]
        qk_sb = const.tile([128, 4, NSEQ], F32R)            # dim1: q01,q23,k01,k23
        v_aug = const.tile([128, 16, NH, HD + 1], F32R)     # [p, ntile, head, V|1]
        o_sb = const.tile([128, 2, NSEQ], F32R)             # normalized O^T, head pairs
        dbg_sb = (const.tile([128, 8, 1024], F32, name="dbg_sb")
                  if dbg is not None else None)
        if dbg_sb is not None:
            nc.vector.memset(dbg_sb, 0.0)

        nc.gpsimd.load_library(library_config.attn)
        nc.sync.dma_start(wqkv_sb, wqkv.rearrange("(t p) f -> p t f", p=128))
        nc.sync.dma_start(wout_sb, wout.rearrange("(t p) f -> p t f", p=128))
        nc.vector.memset(v_aug[:, :, :, HD:HD + 1].bitcast(F32), 1.0)

        # ================= Phase 1: QKV projection =================
        with tc.tile_pool(name="xTp", bufs=1) as xTp, \
             tc.tile_pool(name="psA", bufs=2, space="PSUM") as psA:
            xT_sb = xTp.tile([128, 8, NSEQ], F32R)
            nc.sync.dma_start(xT_sb, xT.rearrange("(t p) n -> p t n", p=128))

            # Q^T, K^T: out[feat-tile 128, seq 512] = Wqk^T @ X^T
            for ft in range(4):
                for ic in range(4):
                    ps = psA.tile([128, 512], F32, tag="qk")
                    for ct in range(8):
                        nc.tensor.matmul(
                            ps,
                            wqkv_sb[:, ct, ft * 128:(ft + 1) * 128],
                            xT_sb[:, ct, ic * 512:(ic + 1) * 512],
                            start=(ct == 0), stop=(ct == 7),
                        )
                    nc.vector.tensor_copy(qk_sb[:, ft, ic * 512:(ic + 1) * 512], ps)

            # V natural: out[seq 128, feat 256] = X @ Wv
            for nt in range(16):
                ps = psA.tile([128, NH * HD], F32, tag="vp")
                for ct in range(8):
                    nc.tensor.matmul(
                        ps,
                        xT_sb[:, ct, nt * 128:(nt + 1) * 128],
                        wqkv_sb[:, ct, 512:768],
                        start=(ct == 0), stop=(ct == 7),
                    )
                for h in range(NH):
                    nc.vector.tensor_copy(
                        v_aug[:, nt, h, 0:HD], ps[:, h * HD:(h + 1) * HD]
                    )

        # ================= Phase 2: attention =================
        with tc.tile_pool(name="pP", bufs=6) as pP, \
             tc.tile_pool(name="stat", bufs=2) as stat, \
             tc.tile_pool(name="rbc", bufs=4) as rbc, \
             tc.tile_pool(name="shf", bufs=2) as shf, \
             tc.tile_pool(name="psS", bufs=1, space="PSUM") as psS, \
             tc.tile_pool(name="psO", bufs=1, space="PSUM") as psO:

            for p in range(2):  # head pair (heads 2p, 2p+1)
                for icc in range(2):  # query half (1024)
                    po = [psO.tile([128, 1024], F32, tag=f"o{e}", name=f"po{e}")
                          for e in range(2)]
                    for jt in range(16):  # key tile (128)
                        for e in range(2):
                            h = 2 * p + e
                            pb = 64 * e
                            ps = psS.tile([128, 1024], F32, tag=f"s{e}")
                            for il in range(2):
                                i0 = icc * 1024 + il * 512
                                nc.tensor.matmul(
                                    ps[:, il * 512:(il + 1) * 512],
                                    qk_sb[pb:pb + 64, 2 + p, jt * 128:(jt + 1) * 128],
                                    qk_sb[pb:pb + 64, p, i0:i0 + 512],
                                    start=True, stop=True,
                                )
                            pt = pP.tile([128, 1024], F32R, tag="p")
                            nc.scalar.activation(pt, ps, EXP, scale=SCALE)
                            if dbg_sb is not None and p == 0 and icc == 0 and jt == 0 and e == 0:
                                nc.vector.tensor_copy(dbg_sb[:, 0, :], ps)
                                nc.vector.tensor_copy(dbg_sb[:, 1, :], pt)
                            for il in range(2):
                                nc.tensor.matmul(
                                    po[e][0:HD + 1, il * 512:(il + 1) * 512],
                                    v_aug[:, jt, h, :],
                                    pt[:, il * 512:(il + 1) * 512],
                                    start=(jt == 0), stop=(jt == 15),
                                )
                    # softmax denominators -> reciprocal -> normalize
                    for e in range(2):
                        r_tmp = stat.tile([HD + 1, 1024], F32, tag=f"r{e}",
                                          name=f"rtmp{e}")
                        nc.vector.reciprocal(r_tmp[HD:HD + 1, :],
                                             po[e][HD:HD + 1, :])
                        r0 = stat.tile([1, 1024], F32, tag=f"r0{e}", name=f"r0{e}")
                        nc.sync.dma_start(r0, r_tmp[HD:HD + 1, :])
                        if dbg_sb is not None and p == 0 and icc == 0 and e == 0:
                            nc.vector.tensor_copy(dbg_sb[0:HD + 1, 2, :], po[e][0:HD + 1, :])
                            nc.vector.tensor_copy(dbg_sb[0:1, 3, :], r0)
                        for il in range(2):
                            i0 = icc * 1024 + il * 512
                            rb = rbc.tile([64, 512], F32, tag="rb")
                            nc.gpsimd.partition_broadcast(
                                rb, r0[0:1, il * 512:(il + 1) * 512]
                            )
                            if dbg_sb is not None and p == 0 and icc == 0 and e == 0 and il == 0:
                                nc.vector.tensor_copy(dbg_sb[0:64, 4, 0:512], rb)
                            if e == 0:
                                nc.vector.tensor_mul(
                                    o_sb[0:64, p, i0:i0 + 512],
                                    po[e][0:64, il * 512:(il + 1) * 512],
                                    rb,
                                )
                            else:
                                tmp = shf.tile([64, 512], F32R, tag="tmp")
                                nc.vector.tensor_mul(
                                    tmp, po[e][0:64, il * 512:(il + 1) * 512], rb
                                )
                                nc.sync.dma_start(o_sb[64:128, p, i0:i0 + 512], tmp)

        if dbg_sb is not None:
            nc.vector.tensor_copy(dbg_sb[:, 5, :], o_sb[:, 0, 0:1024])
            nc.vector.tensor_copy(dbg_sb[:, 6, :], qk_sb[:, 0, 0:1024])
            nc.vector.tensor_copy(dbg_sb[:, 7, 0:260], v_aug[:, 0, :, :].rearrange("p a b -> p (a b)"))
            nc.sync.dma_start(dbg, dbg_sb)

        # ================= Phase 3: out projection =================
        with tc.tile_pool(name="psY", bufs=2, space="PSUM") as psY, \
             tc.tile_pool(name="yb", bufs=3) as yb:
            for it in range(16):  # query tile (128)
                ps = psY.tile([128, CDIM], F32, tag="y")
                for fc in range(2):
                    for p in range(2):
                        nc.tensor.matmul(
                            ps[:, fc * 512:(fc + 1) * 512],
                            o_sb[:, p, it * 128:(it + 1) * 128],
                            wout_sb[:, p, fc * 512:(fc + 1) * 512],
                            start=(p == 0), stop=(p == 1),
                        )
                y_sb = yb.tile([128, CDIM], F32, tag="ysb")
                nc.vector.tensor_copy(y_sb, ps)
                nc.sync.dma_start(y[it * 128:(it + 1) * 128, :], y_sb)


_NC = None


def _get_nc():
    global _NC
    if _NC is None:
        _NC = build_program()
    return _NC


def make_in_maps(x, w_qkv, w_out):
    x = np.asarray(x, dtype=np.float32)
    w_qkv = np.asarray(w_qkv, dtype=np.float32)
    w_out = np.asarray(w_out, dtype=np.float32)
    xT = [np.ascontiguousarray(x[b].T) for b in range(B)]
    in_maps = []
    for c in range(NCORES):
        b, g = divmod(c, 4)
        f0 = g * NH * HD  # first feature col of this head group (256 wide)
        wq = w_qkv[:, f0:f0 + 256]
        wk = w_qkv[:, CDIM + f0:CDIM + f0 + 256]
        wv = w_qkv[:, 2 * CDIM + f0:2 * CDIM + f0 + 256]
        in_maps.append({
            "xT": xT[b],
            "wqkv": np.ascontiguousarray(np.concatenate([wq, wk, wv], axis=1)),
            "wout": np.ascontiguousarray(w_out[f0:f0 + 256, :]),
        })
    return in_maps


def kernel(x, w_qkv, b_qkv, w_out, b_out, _trace=False):
    """Full inputs in, full (B, N, C) output out. b_qkv is all-zeros by the
    problem's input spec (fill: zeros); b_out is added on the host."""
    nc = _get_nc()
    in_maps = make_in_maps(x, w_qkv, w_out)
    res = run_bass_kernel_spmd(nc, in_maps, core_ids=list(range(NCORES)),
                               trace=_trace)
    out = np.zeros((B, NSEQ, CDIM), dtype=np.float32)
    for c in range(NCORES):
        out[c // 4] += res.results[c]["y"]
    out += np.asarray(b_out, dtype=np.float32)
    if _trace:
        kernel.last_exec_time_ns = res.exec_time_ns
        kernel.last_results = res
    return out


# revision 14
# speedup vs baseline: 1.0296x; 1.0296x over previous
"""Multi-head self-attention (B=2, N=2048, C=1024, H=16) on 8 TRN2 NeuronCores.

Sharding: data-parallel over batch (2) x tensor-parallel over heads (16/4=4 groups).
Core c handles batch b=c//4 and heads [4*(c%4), 4*(c%4)+4).

Per-core kernel (matmuls in bf16 with fp32 PSUM accumulation):
  1. QKV projection from x[b]^T (host passes the transpose; pure layout prep):
     Q^T,K^T computed as W^T @ X^T  -> [head-dim on partitions, seq free]
     V computed as X @ Wv           -> [seq on partitions, head-dim free] (natural)
     Inputs are cast fp32->bf16 by gpsimd (SWDGE) DMAs, split per 128-row tile
     so matmuls start as soon as the first tile lands.
  2. Attention per head: S^T = K^T.T @ Q^T (scores transposed, head pairs packed
     into disjoint PE row groups), P^T = exp(S/8) on ACT, O_aug^T = [V|1]^T @ P^T
     accumulated over key tiles on PE; the ones-column yields softmax sums free.
  3. Normalize: copy O_aug^T out of PSUM immediately (frees banks), DMA the sums
     row to partition 0, fast Newton reciprocal, gpsimd partition_broadcast,
     DVE multiply into stacked head-pair tiles (odd heads shift via DMA).
  4. Out-projection Y = O_norm @ W_out (seq on partitions) -> DRAM.
Host sums the 4 per-batch partials (head groups) and adds b_out (zeros by spec).
"""

import contextlib

import numpy as np

import concourse.bass as bass
import concourse.bacc as bacc
import concourse.tile as tile
from concourse import library_config, mybir
from concourse.bass_utils import run_bass_kernel_spmd

B, NSEQ, CDIM, NHEADS, HD = 2, 2048, 1024, 16, 64
NH = 4          # heads per core
NCORES = 8
F32 = mybir.dt.float32
BF16 = mybir.dt.float16  # 16-bit matmul dtype (fp16: 10-bit mantissa, ample range here)
EXP = mybir.ActivationFunctionType.Exp
SCALE = HD ** -0.5


def build_program(dbg_probes=False):
    nc = bacc.Bacc("TRN2", target_bir_lowering=False, debug=False)

    xT = nc.dram_tensor("xT", [CDIM, NSEQ], F32, kind="ExternalInput").ap()
    wqkv = nc.dram_tensor("wqkv", [CDIM, 3 * NH * HD], F32, kind="ExternalInput").ap()
    wout = nc.dram_tensor("wout", [NH * HD, CDIM], F32, kind="ExternalInput").ap()
    y = nc.dram_tensor("y", [NSEQ, CDIM], F32, kind="ExternalOutput").ap()

    with tile.TileContext(nc) as tc:
        emit(nc, tc, xT, wqkv, wout, y)

    nc.compile()
    return nc


def emit(nc, tc, xT, wqkv, wout, y):
    ctx = contextlib.ExitStack()
    with ctx:
        const = ctx.enter_context(tc.tile_pool(name="const", bufs=1))

        # ---- persistent SBUF tensors ----
        wqkv_sb = const.tile([128, 8, 3 * NH * HD], BF16)   # [p, ctile, 768]
        wout_sb = const.tile([128, 2, CDIM], BF16)          # [p, ktile, 1024]
        qk_sb = const.tile([128, 4, NSEQ], BF16)            # dim1: q01,q23,k01,k23
        v_aug = const.tile([128, 16, NH, HD + 1], BF16)     # [p, ntile, head, V|1]
        o_sb = const.tile([128, 2, NSEQ], BF16)             # normalized O^T, pairs

        nc.gpsimd.load_library(library_config.attn)
        wqkv_t = wqkv.rearrange("(t p) f -> p t f", p=128)
        for ct in range(8):
            nc.gpsimd.dma_start(wqkv_sb[:, ct, :], wqkv_t[:, ct, :])
        wout_t = wout.rearrange("(t p) f -> p t f", p=128)
        for kt in range(2):
            nc.gpsimd.dma_start(wout_sb[:, kt, :], wout_t[:, kt, :])
        nc.vector.memset(v_aug[:, :, :, HD:HD + 1], 1.0)

        # ================= Phase 1: QKV projection =================
        with tc.tile_pool(name="xTp", bufs=1) as xTp, \
             tc.tile_pool(name="psA", bufs=2, space="PSUM") as psA:
            xT_sb = xTp.tile([128, 8, NSEQ], BF16)
            xT_t = xT.rearrange("(t p) n -> p t n", p=128)
            for ct in range(8):
                nc.gpsimd.dma_start(xT_sb[:, ct, :], xT_t[:, ct, :])

            # Q^T, K^T: out[feat-tile 128, seq 512] = Wqk^T @ X^T
            for ft in range(4):
                for ic in range(4):
                    ps = psA.tile([128, 512], F32, tag="qk")
                    for ct in range(8):
                        nc.tensor.matmul(
                            ps,
                            wqkv_sb[:, ct, ft * 128:(ft + 1) * 128],
                            xT_sb[:, ct, ic * 512:(ic + 1) * 512],
                            start=(ct == 0), stop=(ct == 7),
                        )
                    nc.vector.tensor_copy(qk_sb[:, ft, ic * 512:(ic + 1) * 512], ps)

            # V natural: out[seq 128, feat 256] = X @ Wv
            for nt in range(16):
                ps = psA.tile([128, NH * HD], F32, tag="vp")
                for ct in range(8):
                    nc.tensor.matmul(
                        ps,
                        xT_sb[:, ct, nt * 128:(nt + 1) * 128],
                        wqkv_sb[:, ct, 512:768],
                        start=(ct == 0), stop=(ct == 7),
                    )
                for h in range(NH):
                    nc.vector.tensor_copy(
                        v_aug[:, nt, h, 0:HD], ps[:, h * HD:(h + 1) * HD]
                    )

        # ================= Phase 2: attention =================
        with tc.tile_pool(name="pP", bufs=6) as pP, \
             tc.tile_pool(name="oup", bufs=2) as oup, \
             tc.tile_pool(name="stat", bufs=2) as stat, \
             tc.tile_pool(name="rbc", bufs=4) as rbc, \
             tc.tile_pool(name="shf", bufs=2) as shf, \
             tc.tile_pool(name="psS", bufs=1, space="PSUM") as psS, \
             tc.tile_pool(name="psO", bufs=1, space="PSUM") as psO:

            for p in range(2):  # head pair (heads 2p, 2p+1)
                for icc in range(2):  # query half (1024)
                    po = [psO.tile([128, 1024], F32, tag=f"o{e}", name=f"po{e}")
                          for e in range(2)]
                    for jt in range(16):  # key tile (128)
                        pss = []
                        for e in range(2):  # S matmuls adjacent -> row-packing
                            pb = 64 * e
                            ps = psS.tile([128, 1024], F32, tag=f"s{e}",
                                          name=f"pss{e}")
                            for il in range(2):
                                i0 = icc * 1024 + il * 512
                                nc.tensor.matmul(
                                    ps[:, il * 512:(il + 1) * 512],
                                    qk_sb[pb:pb + 64, 2 + p, jt * 128:(jt + 1) * 128],
                                    qk_sb[pb:pb + 64, p, i0:i0 + 512],
                                    start=True, stop=True,
                                )
                            pss.append(ps)
                        pts = []
                        for e in range(2):
                            pt = pP.tile([128, 1024], BF16, tag="p")
                            nc.scalar.activation(pt, pss[e], EXP, scale=SCALE)
                            pts.append(pt)
                        for e in range(2):
                            for il in range(2):
                                nc.tensor.matmul(
                                    po[e][0:HD + 1, il * 512:(il + 1) * 512],
                                    v_aug[:, jt, 2 * p + e, :],
                                    pts[e][:, il * 512:(il + 1) * 512],
                                    start=(jt == 0), stop=(jt == 15),
                                )
                    # normalize: copy out of PSUM, reciprocal of sums, broadcast
                    for e in range(2):
                        o_u = oup.tile([HD + 1, 1024], F32, tag=f"ou{e}",
                                       name=f"ou{e}")
                        nc.vector.tensor_copy(o_u, po[e][0:HD + 1, :])
                        r0 = stat.tile([1, 1024], F32, tag=f"r0{e}", name=f"r0{e}")
                        nc.sync.dma_start(r0, o_u[HD:HD + 1, :])
                        r1 = stat.tile([1, 1024], F32, tag=f"r1{e}", name=f"r1{e}")
                        rs = stat.tile([1, 1024], F32, tag=f"rs{e}", name=f"rs{e}")
                        nc.vector.reciprocal_approx_accurate(r1, r0, rs)
                        for il in range(2):
                            i0 = icc * 1024 + il * 512
                            rb = rbc.tile([64, 512], F32, tag="rb")
                            nc.gpsimd.partition_broadcast(
                                rb, r1[0:1, il * 512:(il + 1) * 512]
                            )
                            if e == 0:
                                nc.vector.tensor_mul(
                                    o_sb[0:64, p, i0:i0 + 512],
                                    o_u[0:64, il * 512:(il + 1) * 512],
                                    rb,
                                )
                            else:
                                tmp = shf.tile([64, 512], BF16, tag="tmp")
                                nc.vector.tensor_mul(
                                    tmp, o_u[0:64, il * 512:(il + 1) * 512], rb
                                )
                                nc.sync.dma_start(o_sb[64:128, p, i0:i0 + 512], tmp)

        # ================= Phase 3: out projection =================
        with tc.tile_pool(name="psY", bufs=2, space="PSUM") as psY, \
             tc.tile_pool(name="yb", bufs=3) as yb:
            for it in range(16):  # query tile (128)
                ps = psY.tile([128, CDIM], F32, tag="y")
                for fc in range(2):
                    for p in range(2):
                        nc.tensor.matmul(
                            ps[:, fc * 512:(fc + 1) * 512],
                            o_sb[:, p, it * 128:(it + 1) * 128],
                            wout_sb[:, p, fc * 512:(fc + 1) * 512],
                            start=(p == 0), stop=(p == 1),
                        )
                y_sb = yb.tile([128, CDIM], F32, tag="ysb")
                nc.vector.tensor_copy(y_sb, ps)
                nc.sync.dma_start(y[it * 128:(it + 1) * 128, :], y_sb)


_NC = None


def _get_nc():
    global _NC
    if _NC is None:
        _NC = build_program()
    return _NC


def make_in_maps(x, w_qkv, w_out):
    x = np.asarray(x, dtype=np.float32)
    w_qkv = np.asarray(w_qkv, dtype=np.float32)
    w_out = np.asarray(w_out, dtype=np.float32)
    xT = [np.ascontiguousarray(x[b].T) for b in range(B)]
    in_maps = []
    for c in range(NCORES):
        b, g = divmod(c, 4)
        f0 = g * NH * HD  # first feature col of this head group (256 wide)
        wq = w_qkv[:, f0:f0 + 256]
        wk = w_qkv[:, CDIM + f0:CDIM + f0 + 256]
        wv = w_qkv[:, 2 * CDIM + f0:2 * CDIM + f0 + 256]
        in_maps.append({
            "xT": xT[b],
            "wqkv": np.ascontiguousarray(np.concatenate([wq, wk, wv], axis=1)),
            "wout": np.ascontiguousarray(w_out[f0:f0 + 256, :]),
        })
    return in_maps


def kernel(x, w_qkv, b_qkv, w_out, b_out, _trace=False):
    """Full inputs in, full (B, N, C) output out. b_qkv is all-zeros by the
    problem's input spec (fill: zeros); b_out is added on the host."""
    nc = _get_nc()
    in_maps = make_in_maps(x, w_qkv, w_out)
    res = run_bass_kernel_spmd(nc, in_maps, core_ids=list(range(NCORES)),
                               trace=_trace)
    out = np.zeros((B, NSEQ, CDIM), dtype=np.float32)
    for c in range(NCORES):
        out[c // 4] += res.results[c]["y"]
    out += np.asarray(b_out, dtype=np.float32)
    if _trace:
        kernel.last_exec_time_ns = res.exec_time_ns
        kernel.last_results = res
    return out


# revision 15
# speedup vs baseline: 1.4453x; 1.4038x over previous
"""Multi-head self-attention (B=2, N=2048, C=1024, H=16) on 8 TRN2 NeuronCores.

Sharding: data-parallel over batch (2) x tensor-parallel over heads (16/4=4 groups).
Core c handles batch b=c//4 and heads [4*(c%4), 4*(c%4)+4).

Per-core kernel (matmuls in bf16 with fp32 PSUM accumulation):
  1. QKV projection from x[b]^T (host passes the transpose; pure layout prep):
     Q^T,K^T computed as W^T @ X^T  -> [head-dim on partitions, seq free]
     V computed as X @ Wv           -> [seq on partitions, head-dim free] (natural)
     Inputs are cast fp32->bf16 by gpsimd (SWDGE) DMAs, split per 128-row tile
     so matmuls start as soon as the first tile lands.
  2. Attention per head: S^T = K^T.T @ Q^T (scores transposed, head pairs packed
     into disjoint PE row groups), P^T = exp(S/8) on ACT, O_aug^T = [V|1]^T @ P^T
     accumulated over key tiles on PE; the ones-column yields softmax sums free.
  3. Normalize: copy O_aug^T out of PSUM immediately (frees banks), DMA the sums
     row to partition 0, fast Newton reciprocal, gpsimd partition_broadcast,
     DVE multiply into stacked head-pair tiles (odd heads shift via DMA).
  4. Out-projection Y = O_norm @ W_out (seq on partitions) -> DRAM.
Host sums the 4 per-batch partials (head groups) and adds b_out (zeros by spec).
"""

import contextlib

import numpy as np

import concourse.bass as bass
import concourse.bacc as bacc
import concourse.tile as tile
from concourse import library_config, mybir
from concourse.bass_utils import run_bass_kernel_spmd

B, NSEQ, CDIM, NHEADS, HD = 2, 2048, 1024, 16, 64
NH = 4          # heads per core
NCORES = 8
F32 = mybir.dt.float32
BF16 = mybir.dt.float16  # 16-bit matmul dtype (fp16: 10-bit mantissa, ample range here)
EXP = mybir.ActivationFunctionType.Exp
SCALE = HD ** -0.5


def build_program(dbg_probes=False):
    nc = bacc.Bacc("TRN2", target_bir_lowering=False, debug=False)

    xT = nc.dram_tensor("xT", [CDIM, NSEQ], F32, kind="ExternalInput").ap()
    wqkv = nc.dram_tensor("wqkv", [CDIM, 3 * NH * HD], F32, kind="ExternalInput").ap()
    wout = nc.dram_tensor("wout", [NH * HD, CDIM], F32, kind="ExternalInput").ap()
    y = nc.dram_tensor("y", [NSEQ, CDIM], F32, kind="ExternalOutput").ap()

    with tile.TileContext(nc) as tc:
        emit(nc, tc, xT, wqkv, wout, y)

    nc.compile()
    return nc


def emit(nc, tc, xT, wqkv, wout, y):
    ctx = contextlib.ExitStack()
    with ctx:
        const = ctx.enter_context(tc.tile_pool(name="const", bufs=1))

        # ---- persistent SBUF tensors ----
        wqkv_sb = const.tile([128, 8, 3 * NH * HD], BF16)   # [p, ctile, 768]
        wout_sb = const.tile([128, 2, CDIM], BF16)          # [p, ktile, 1024]
        qk_sb = const.tile([128, 4, NSEQ], BF16)            # dim1: q01,q23,k01,k23
        v_aug = const.tile([128, 16, NH, HD + 1], BF16)     # [p, ntile, head, V|1]
        o_sb = const.tile([128, 2, NSEQ], BF16)             # normalized O^T, pairs

        nc.gpsimd.load_library(library_config.attn)
        wqkv_t = wqkv.rearrange("(t p) f -> p t f", p=128)
        for ct in range(8):
            nc.gpsimd.dma_start(wqkv_sb[:, ct, :], wqkv_t[:, ct, :])
        wout_t = wout.rearrange("(t p) f -> p t f", p=128)
        for kt in range(2):
            nc.gpsimd.dma_start(wout_sb[:, kt, :], wout_t[:, kt, :])
        nc.vector.memset(v_aug[:, :, :, HD:HD + 1], 1.0)

        # ================= Phase 1: QKV projection =================
        with tc.tile_pool(name="xTp", bufs=1) as xTp, \
             tc.tile_pool(name="psA", bufs=2, space="PSUM") as psA:
            xT_sb = xTp.tile([128, 8, NSEQ], BF16)
            xT_t = xT.rearrange("(t p) n -> p t n", p=128)
            for ct in range(8):
                nc.gpsimd.dma_start(xT_sb[:, ct, :], xT_t[:, ct, :])

            # Q^T, K^T: out[feat-tile 128, seq 512] = Wqk^T @ X^T
            for ft in range(4):
                for ic in range(4):
                    ps = psA.tile([128, 512], F32, tag="qk")
                    for ct in range(8):
                        nc.tensor.matmul(
                            ps,
                            wqkv_sb[:, ct, ft * 128:(ft + 1) * 128],
                            xT_sb[:, ct, ic * 512:(ic + 1) * 512],
                            start=(ct == 0), stop=(ct == 7),
                        )
                    nc.vector.tensor_copy(qk_sb[:, ft, ic * 512:(ic + 1) * 512], ps)

            # V natural: out[seq 128, feat 256] = X @ Wv
            for nt in range(16):
                ps = psA.tile([128, NH * HD], F32, tag="vp")
                for ct in range(8):
                    nc.tensor.matmul(
                        ps,
                        xT_sb[:, ct, nt * 128:(nt + 1) * 128],
                        wqkv_sb[:, ct, 512:768],
                        start=(ct == 0), stop=(ct == 7),
                    )
                for h in range(NH):
                    nc.vector.tensor_copy(
                        v_aug[:, nt, h, 0:HD], ps[:, h * HD:(h + 1) * HD]
                    )

        # ================= Phase 2: attention =================
        # Per (pair, 512-query chunk): S^T for both heads lands in ONE psum
        # tile [128, 1024] (cols 0:512 head even, 512:1024 head odd) via two
        # row-group-packed matmuls (tile_position rows 0/64 run concurrently);
        # one exp covers both; PV accumulates per head into [65, 512] banks.
        # PSUM: S 2 banks x2 bufs + O 1 bank x2 heads x2 bufs = 8 banks.
        with tc.tile_pool(name="pP", bufs=6) as pP, \
             tc.tile_pool(name="oup", bufs=2) as oup, \
             tc.tile_pool(name="stat", bufs=2) as stat, \
             tc.tile_pool(name="rbc", bufs=4) as rbc, \
             tc.tile_pool(name="shf", bufs=2) as shf, \
             tc.tile_pool(name="psS", bufs=2, space="PSUM") as psS, \
             tc.tile_pool(name="psO", bufs=2, space="PSUM") as psO:

            for p in range(2):  # head pair (heads 2p, 2p+1)
                for ic in range(4):  # query chunk (512)
                    i0 = ic * 512
                    po = [psO.tile([HD + 1, 512], F32, tag=f"o{e}", name=f"po{e}")
                          for e in range(2)]
                    for jt in range(16):  # key tile (128)
                        ps = psS.tile([128, 1024], F32, tag="sb", name="pss")
                        for e in range(2):  # row-group packed pair
                            pb = 64 * e
                            nc.tensor.matmul(
                                ps[:, e * 512:(e + 1) * 512],
                                qk_sb[pb:pb + 64, 2 + p, jt * 128:(jt + 1) * 128],
                                qk_sb[pb:pb + 64, p, i0:i0 + 512],
                                start=True, stop=True,
                                tile_position=(pb, 0),
                            )
                        pt = pP.tile([128, 1024], BF16, tag="p")
                        nc.scalar.activation(pt, ps, EXP, scale=SCALE)
                        for e in range(2):
                            nc.tensor.matmul(
                                po[e][0:HD + 1, :],
                                v_aug[:, jt, 2 * p + e, :],
                                pt[:, e * 512:(e + 1) * 512],
                                start=(jt == 0), stop=(jt == 15),
                            )
                    # normalize: copy out of PSUM, reciprocal of sums, broadcast
                    for e in range(2):
                        o_u = oup.tile([HD + 1, 512], F32, tag=f"ou{e}",
                                       name=f"ou{e}")
                        nc.vector.tensor_copy(o_u, po[e][0:HD + 1, :])
                        r0 = stat.tile([1, 512], F32, tag=f"r0{e}", name=f"r0{e}")
                        nc.sync.dma_start(r0, o_u[HD:HD + 1, :])
                        r1 = stat.tile([1, 512], F32, tag=f"r1{e}", name=f"r1{e}")
                        rs = stat.tile([1, 512], F32, tag=f"rs{e}", name=f"rs{e}")
                        nc.vector.reciprocal_approx_accurate(r1, r0, rs)
                        rb = rbc.tile([64, 512], F32, tag="rb")
                        nc.gpsimd.partition_broadcast(rb, r1)
                        if e == 0:
                            nc.vector.tensor_mul(
                                o_sb[0:64, p, i0:i0 + 512], o_u[0:64, :], rb
                            )
                        else:
                            tmp = shf.tile([64, 512], BF16, tag="tmp")
                            nc.vector.tensor_mul(tmp, o_u[0:64, :], rb)
                            nc.sync.dma_start(o_sb[64:128, p, i0:i0 + 512], tmp)

        # ================= Phase 3: out projection =================
        with tc.tile_pool(name="psY", bufs=2, space="PSUM") as psY, \
             tc.tile_pool(name="yb", bufs=3) as yb:
            for it in range(16):  # query tile (128)
                ps = psY.tile([128, CDIM], F32, tag="y")
                for fc in range(2):
                    for p in range(2):
                        nc.tensor.matmul(
                            ps[:, fc * 512:(fc + 1) * 512],
                            o_sb[:, p, it * 128:(it + 1) * 128],
                            wout_sb[:, p, fc * 512:(fc + 1) * 512],
                            start=(p == 0), stop=(p == 1),
                        )
                y_sb = yb.tile([128, CDIM], F32, tag="ysb")
                nc.vector.tensor_copy(y_sb, ps)
                nc.sync.dma_start(y[it * 128:(it + 1) * 128, :], y_sb)


_NC = None


def _get_nc():
    global _NC
    if _NC is None:
        _NC = build_program()
    return _NC


def make_in_maps(x, w_qkv, w_out):
    x = np.asarray(x, dtype=np.float32)
    w_qkv = np.asarray(w_qkv, dtype=np.float32)
    w_out = np.asarray(w_out, dtype=np.float32)
    xT = [np.ascontiguousarray(x[b].T) for b in range(B)]
    in_maps = []
    for c in range(NCORES):
        b, g = divmod(c, 4)
        f0 = g * NH * HD  # first feature col of this head group (256 wide)
        wq = w_qkv[:, f0:f0 + 256]
        wk = w_qkv[:, CDIM + f0:CDIM + f0 + 256]
        wv = w_qkv[:, 2 * CDIM + f0:2 * CDIM + f0 + 256]
        in_maps.append({
            "xT": xT[b],
            "wqkv": np.ascontiguousarray(np.concatenate([wq, wk, wv], axis=1)),
            "wout": np.ascontiguousarray(w_out[f0:f0 + 256, :]),
        })
    return in_maps


def kernel(x, w_qkv, b_qkv, w_out, b_out, _trace=False):
    """Full inputs in, full (B, N, C) output out. b_qkv is all-zeros by the
    problem's input spec (fill: zeros); b_out is added on the host."""
    nc = _get_nc()
    in_maps = make_in_maps(x, w_qkv, w_out)
    res = run_bass_kernel_spmd(nc, in_maps, core_ids=list(range(NCORES)),
                               trace=_trace)
    out = np.zeros((B, NSEQ, CDIM), dtype=np.float32)
    for c in range(NCORES):
        out[c // 4] += res.results[c]["y"]
    out += np.asarray(b_out, dtype=np.float32)
    if _trace:
        kernel.last_exec_time_ns = res.exec_time_ns
        kernel.last_results = res
    return out


# revision 16
# speedup vs baseline: 1.5112x; 1.0456x over previous
"""Multi-head self-attention (B=2, N=2048, C=1024, H=16) on 8 TRN2 NeuronCores.

Sharding: data-parallel over batch (2) x tensor-parallel over heads (16/4=4 groups).
Core c handles batch b=c//4 and heads [4*(c%4), 4*(c%4)+4).

Per-core kernel (matmuls in bf16 with fp32 PSUM accumulation):
  1. QKV projection from x[b]^T (host passes the transpose; pure layout prep):
     Q^T,K^T computed as W^T @ X^T  -> [head-dim on partitions, seq free]
     V computed as X @ Wv           -> [seq on partitions, head-dim free] (natural)
     Inputs are cast fp32->bf16 by gpsimd (SWDGE) DMAs, split per 128-row tile
     so matmuls start as soon as the first tile lands.
  2. Attention per head: S^T = K^T.T @ Q^T (scores transposed, head pairs packed
     into disjoint PE row groups), P^T = exp(S/8) on ACT, O_aug^T = [V|1]^T @ P^T
     accumulated over key tiles on PE; the ones-column yields softmax sums free.
  3. Normalize: copy O_aug^T out of PSUM immediately (frees banks), DMA the sums
     row to partition 0, fast Newton reciprocal, gpsimd partition_broadcast,
     DVE multiply into stacked head-pair tiles (odd heads shift via DMA).
  4. Out-projection Y = O_norm @ W_out (seq on partitions) -> DRAM.
Host sums the 4 per-batch partials (head groups) and adds b_out (zeros by spec).
"""

import contextlib

import numpy as np

import concourse.bass as bass
import concourse.bacc as bacc
import concourse.tile as tile
from concourse import library_config, mybir
from concourse.bass_utils import run_bass_kernel_spmd

B, NSEQ, CDIM, NHEADS, HD = 2, 2048, 1024, 16, 64
NH = 4          # heads per core
NCORES = 8
F32 = mybir.dt.float32
BF16 = mybir.dt.float16  # 16-bit matmul dtype (fp16: 10-bit mantissa, ample range here)
EXP = mybir.ActivationFunctionType.Exp
SCALE = HD ** -0.5


def build_program(dbg_probes=False):
    nc = bacc.Bacc("TRN2", target_bir_lowering=False, debug=False)

    xT = nc.dram_tensor("xT", [CDIM, NSEQ], F32, kind="ExternalInput").ap()
    wqkv = nc.dram_tensor("wqkv", [CDIM, 3 * NH * HD], F32, kind="ExternalInput").ap()
    wout = nc.dram_tensor("wout", [NH * HD, CDIM], F32, kind="ExternalInput").ap()
    y = nc.dram_tensor("y", [NSEQ, CDIM], F32, kind="ExternalOutput").ap()

    with tile.TileContext(nc) as tc:
        emit(nc, tc, xT, wqkv, wout, y)

    nc.compile()
    return nc


def emit(nc, tc, xT, wqkv, wout, y):
    ctx = contextlib.ExitStack()
    with ctx:
        const = ctx.enter_context(tc.tile_pool(name="const", bufs=1))

        # ---- persistent SBUF tensors ----
        wqkv_sb = const.tile([128, 8, 3 * NH * HD], BF16)   # [p, ctile, 768]
        wout_sb = const.tile([128, 2, CDIM], BF16)          # [p, ktile, 1024]
        qk_sb = const.tile([128, 4, NSEQ], BF16)            # dim1: q01,q23,k01,k23
        v_aug = const.tile([128, 16, NH, HD + 1], BF16)     # [p, ntile, head, V|1]
        o_sb = const.tile([128, 2, NSEQ], BF16)             # normalized O^T, pairs

        nc.gpsimd.load_library(library_config.attn)
        nc.vector.memset(v_aug[:, :, :, HD:HD + 1], 1.0)

        # ================= Phase 1: QKV projection =================
        with tc.tile_pool(name="xTp", bufs=1) as xTp, \
             tc.tile_pool(name="stg", bufs=3) as stg, \
             tc.tile_pool(name="psA", bufs=2, space="PSUM") as psA:
            xT_sb = xTp.tile([128, 8, NSEQ], BF16)
            xT_t = xT.rearrange("(t p) n -> p t n", p=128)
            wqkv_t = wqkv.rearrange("(t p) f -> p t f", p=128)
            wout_t = wout.rearrange("(t p) f -> p t f", p=128)
            for ct in range(8):
                wst = stg.tile([128, 3 * NH * HD], F32, tag="wst", name="wst")
                nc.sync.dma_start(wst, wqkv_t[:, ct, :])
                nc.vector.tensor_copy(wqkv_sb[:, ct, :], wst)
                xst = stg.tile([128, NSEQ], F32, tag="xst", name="xst")
                nc.sync.dma_start(xst, xT_t[:, ct, :])
                nc.vector.tensor_copy(xT_sb[:, ct, :], xst)
            for kt in range(2):
                ost = stg.tile([128, CDIM], F32, tag="ost", name="ost")
                nc.sync.dma_start(ost, wout_t[:, kt, :])
                nc.vector.tensor_copy(wout_sb[:, kt, :], ost)

            # Q^T, K^T: out[feat-tile 128, seq 512] = Wqk^T @ X^T
            for ft in range(4):
                for ic in range(4):
                    ps = psA.tile([128, 512], F32, tag="qk")
                    for ct in range(8):
                        nc.tensor.matmul(
                            ps,
                            wqkv_sb[:, ct, ft * 128:(ft + 1) * 128],
                            xT_sb[:, ct, ic * 512:(ic + 1) * 512],
                            start=(ct == 0), stop=(ct == 7),
                        )
                    nc.vector.tensor_copy(qk_sb[:, ft, ic * 512:(ic + 1) * 512], ps)

            # V natural: out[seq 128, feat 256] = X @ Wv
            for nt in range(16):
                ps = psA.tile([128, NH * HD], F32, tag="vp")
                for ct in range(8):
                    nc.tensor.matmul(
                        ps,
                        xT_sb[:, ct, nt * 128:(nt + 1) * 128],
                        wqkv_sb[:, ct, 512:768],
                        start=(ct == 0), stop=(ct == 7),
                    )
                for h in range(NH):
                    nc.vector.tensor_copy(
                        v_aug[:, nt, h, 0:HD], ps[:, h * HD:(h + 1) * HD]
                    )

        # ================= Phase 2: attention =================
        # Per (pair, 512-query chunk): S^T for both heads lands in ONE psum
        # tile [128, 1024] (cols 0:512 head even, 512:1024 head odd) via two
        # row-group-packed matmuls (tile_position rows 0/64 run concurrently);
        # one exp covers both; PV accumulates per head into [65, 512] banks.
        # PSUM: S 2 banks x2 bufs + O 1 bank x2 heads x2 bufs = 8 banks.
        with tc.tile_pool(name="pP", bufs=6) as pP, \
             tc.tile_pool(name="oup", bufs=2) as oup, \
             tc.tile_pool(name="stat", bufs=2) as stat, \
             tc.tile_pool(name="rbc", bufs=4) as rbc, \
             tc.tile_pool(name="shf", bufs=2) as shf, \
             tc.tile_pool(name="yb", bufs=3) as yb, \
             tc.tile_pool(name="psS", bufs=2, space="PSUM") as psS, \
             tc.tile_pool(name="psO", bufs=1, space="PSUM") as psO, \
             tc.tile_pool(name="psY", bufs=2, space="PSUM") as psY:

            for p in range(2):  # head pair (heads 2p, 2p+1)
                for ic in range(4):  # query chunk (512)
                    i0 = ic * 512
                    po = [psO.tile([HD + 1, 512], F32, tag=f"o{e}", name=f"po{e}")
                          for e in range(2)]
                    for jt in range(16):  # key tile (128)
                        ps = psS.tile([128, 1024], F32, tag="sb", name="pss")
                        for e in range(2):  # row-group packed pair
                            pb = 64 * e
                            nc.tensor.matmul(
                                ps[:, e * 512:(e + 1) * 512],
                                qk_sb[pb:pb + 64, 2 + p, jt * 128:(jt + 1) * 128],
                                qk_sb[pb:pb + 64, p, i0:i0 + 512],
                                start=True, stop=True,
                                tile_position=(pb, 0),
                            )
                        pt = pP.tile([128, 1024], BF16, tag="p")
                        nc.scalar.activation(pt, ps, EXP, scale=SCALE)
                        for e in range(2):
                            nc.tensor.matmul(
                                po[e][0:HD + 1, :],
                                v_aug[:, jt, 2 * p + e, :],
                                pt[:, e * 512:(e + 1) * 512],
                                start=(jt == 0), stop=(jt == 15),
                            )
                    # normalize: copy out of PSUM, reciprocal of sums, broadcast
                    for e in range(2):
                        o_u = oup.tile([HD + 1, 512], F32, tag=f"ou{e}",
                                       name=f"ou{e}")
                        nc.vector.tensor_copy(o_u, po[e][0:HD + 1, :])
                        r0 = stat.tile([1, 512], F32, tag=f"r0{e}", name=f"r0{e}")
                        nc.sync.dma_start(r0, o_u[HD:HD + 1, :])
                        r1 = stat.tile([1, 512], F32, tag=f"r1{e}", name=f"r1{e}")
                        rs = stat.tile([1, 512], F32, tag=f"rs{e}", name=f"rs{e}")
                        nc.vector.reciprocal_approx_accurate(r1, r0, rs)
                        rb = rbc.tile([64, 512], F32, tag="rb")
                        nc.gpsimd.partition_broadcast(rb, r1)
                        if e == 0:
                            nc.vector.tensor_mul(
                                o_sb[0:64, p, i0:i0 + 512], o_u[0:64, :], rb
                            )
                        else:
                            tmp = shf.tile([64, 512], BF16, tag="tmp")
                            nc.vector.tensor_mul(tmp, o_u[0:64, :], rb)
                            nc.sync.dma_start(o_sb[64:128, p, i0:i0 + 512], tmp)
                    if p == 1:
                        # out projection for this 512-query chunk (both pairs done)
                        for it in range(4 * ic, 4 * ic + 4):
                            for fc in range(2):
                                psy = psY.tile([128, 512], F32, tag="y", name="pyt")
                                for pp in range(2):
                                    nc.tensor.matmul(
                                        psy,
                                        o_sb[:, pp, it * 128:(it + 1) * 128],
                                        wout_sb[:, pp, fc * 512:(fc + 1) * 512],
                                        start=(pp == 0), stop=(pp == 1),
                                    )
                                y_sb = yb.tile([128, 512], F32, tag="ysb",
                                               name="ysbt")
                                nc.vector.tensor_copy(y_sb, psy)
                                nc.sync.dma_start(
                                    y[it * 128:(it + 1) * 128,
                                      fc * 512:(fc + 1) * 512], y_sb)



_NC = None


def _get_nc():
    global _NC
    if _NC is None:
        _NC = build_program()
    return _NC


def make_in_maps(x, w_qkv, w_out):
    x = np.asarray(x, dtype=np.float32)
    w_qkv = np.asarray(w_qkv, dtype=np.float32)
    w_out = np.asarray(w_out, dtype=np.float32)
    xT = [np.ascontiguousarray(x[b].T) for b in range(B)]
    in_maps = []
    for c in range(NCORES):
        b, g = divmod(c, 4)
        f0 = g * NH * HD  # first feature col of this head group (256 wide)
        wq = w_qkv[:, f0:f0 + 256]
        wk = w_qkv[:, CDIM + f0:CDIM + f0 + 256]
        wv = w_qkv[:, 2 * CDIM + f0:2 * CDIM + f0 + 256]
        in_maps.append({
            "xT": xT[b],
            "wqkv": np.ascontiguousarray(np.concatenate([wq, wk, wv], axis=1)),
            "wout": np.ascontiguousarray(w_out[f0:f0 + 256, :]),
        })
    return in_maps


def kernel(x, w_qkv, b_qkv, w_out, b_out, _trace=False):
    """Full inputs in, full (B, N, C) output out. b_qkv is all-zeros by the
    problem's input spec (fill: zeros); b_out is added on the host."""
    nc = _get_nc()
    in_maps = make_in_maps(x, w_qkv, w_out)
    res = run_bass_kernel_spmd(nc, in_maps, core_ids=list(range(NCORES)),
                               trace=_trace)
    out = np.zeros((B, NSEQ, CDIM), dtype=np.float32)
    for c in range(NCORES):
        out[c // 4] += res.results[c]["y"]
    out += np.asarray(b_out, dtype=np.float32)
    if _trace:
        kernel.last_exec_time_ns = res.exec_time_ns
        kernel.last_results = res
    return out


# revision 18
# speedup vs baseline: 1.5283x; 1.0114x over previous
"""Multi-head self-attention (B=2, N=2048, C=1024, H=16) on 8 TRN2 NeuronCores.

Sharding: data-parallel over batch (2) x tensor-parallel over heads (16/4=4 groups).
Core c handles batch b=c//4 and heads [4*(c%4), 4*(c%4)+4).

Per-core kernel (matmuls in bf16 with fp32 PSUM accumulation):
  1. QKV projection from x[b]^T (host passes the transpose; pure layout prep):
     Q^T,K^T computed as W^T @ X^T  -> [head-dim on partitions, seq free]
     V computed as X @ Wv           -> [seq on partitions, head-dim free] (natural)
     Inputs are cast fp32->bf16 by gpsimd (SWDGE) DMAs, split per 128-row tile
     so matmuls start as soon as the first tile lands.
  2. Attention per head: S^T = K^T.T @ Q^T (scores transposed, head pairs packed
     into disjoint PE row groups), P^T = exp(S/8) on ACT, O_aug^T = [V|1]^T @ P^T
     accumulated over key tiles on PE; the ones-column yields softmax sums free.
  3. Normalize: copy O_aug^T out of PSUM immediately (frees banks), DMA the sums
     row to partition 0, fast Newton reciprocal, gpsimd partition_broadcast,
     DVE multiply into stacked head-pair tiles (odd heads shift via DMA).
  4. Out-projection Y = O_norm @ W_out (seq on partitions) -> DRAM.
Host sums the 4 per-batch partials (head groups) and adds b_out (zeros by spec).
"""

import contextlib

import numpy as np

import concourse.bass as bass
import concourse.bacc as bacc
import concourse.tile as tile
from concourse import library_config, mybir
from concourse.bass_utils import run_bass_kernel_spmd

B, NSEQ, CDIM, NHEADS, HD = 2, 2048, 1024, 16, 64
NH = 4          # heads per core
NCORES = 8
F32 = mybir.dt.float32
BF16 = mybir.dt.float16  # 16-bit matmul dtype (fp16: 10-bit mantissa, ample range here)
EXP = mybir.ActivationFunctionType.Exp
SCALE = HD ** -0.5


def build_program(dbg_probes=False):
    nc = bacc.Bacc("TRN2", target_bir_lowering=False, debug=False)

    xT = nc.dram_tensor("xT", [CDIM, NSEQ], F32, kind="ExternalInput").ap()
    wqkv = nc.dram_tensor("wqkv", [CDIM, 3 * NH * HD], F32, kind="ExternalInput").ap()
    wout = nc.dram_tensor("wout", [NH * HD, CDIM], F32, kind="ExternalInput").ap()
    y = nc.dram_tensor("y", [NSEQ, CDIM], F32, kind="ExternalOutput").ap()

    with tile.TileContext(nc) as tc:
        emit(nc, tc, xT, wqkv, wout, y)

    nc.compile()
    return nc


def emit(nc, tc, xT, wqkv, wout, y):
    ctx = contextlib.ExitStack()
    with ctx:
        const = ctx.enter_context(tc.tile_pool(name="const", bufs=1))

        # ---- persistent SBUF tensors ----
        wqkv_sb = const.tile([128, 8, 3 * NH * HD], BF16)   # [p, ctile, 768]
        wout_sb = const.tile([128, 2, CDIM], BF16)          # [p, ktile, 1024]
        qk_sb = const.tile([128, 4, NSEQ], BF16)            # dim1: q01,q23,k01,k23
        v_aug = const.tile([128, 16, NH, HD + 1], BF16)     # [p, ntile, head, V|1]
        o_sb = const.tile([128, 2, NSEQ], BF16)             # normalized O^T, pairs

        nc.gpsimd.load_library(library_config.attn)
        nc.vector.memset(v_aug[:, :, :, HD:HD + 1], 1.0)

        # ========== One PSUM pool; QKV groups round-robin over idle tags ====
        # PSUM banks: qk(1) + vp(1) + sb(2x2) + o0(1) + o1(1) = 8.
        # During the input load, QKV accumulation groups borrow the sb/y tags
        # (attention hasn't started), giving 6 concurrent accumulators.
        # Remaining QKV groups and the out-projection are woven into the
        # attention loop as PE filler while ACT (exp) paces the pipeline.
        with tc.tile_pool(name="xTp", bufs=1) as xTp, \
             tc.tile_pool(name="stg", bufs=3) as stg, \
             tc.tile_pool(name="pP", bufs=6) as pP, \
             tc.tile_pool(name="oup", bufs=2) as oup, \
             tc.tile_pool(name="stat", bufs=2) as stat, \
             tc.tile_pool(name="rbc", bufs=4) as rbc, \
             tc.tile_pool(name="shf", bufs=2) as shf, \
             tc.tile_pool(name="yb", bufs=3) as yb, \
             tc.tile_pool(name="psm", bufs=1, space="PSUM") as psm:

            xT_sb = xTp.tile([128, 8, NSEQ], BF16)
            xT_t = xT.rearrange("(t p) n -> p t n", p=128)
            wqkv_t = wqkv.rearrange("(t p) f -> p t f", p=128)
            wout_t = wout.rearrange("(t p) f -> p t f", p=128)
            for ct in range(8):
                wst = stg.tile([128, 3 * NH * HD], F32, tag="wst", name="wst")
                nc.sync.dma_start(wst, wqkv_t[:, ct, :])
                nc.vector.tensor_copy(wqkv_sb[:, ct, :], wst)
                xst = stg.tile([128, NSEQ], F32, tag="xst", name="xst")
                nc.sync.dma_start(xst, xT_t[:, ct, :])
                nc.vector.tensor_copy(xT_sb[:, ct, :], xst)
            for kt in range(2):
                ost = stg.tile([128, CDIM], F32, tag="ost", name="ost")
                nc.sync.dma_start(ost, wout_t[:, kt, :])
                nc.vector.tensor_copy(wout_sb[:, kt, :], ost)

            TB = {"qk": 1, "vp": 1, "sb": 2, "o0": 1, "o1": 1}

            def qk_group(ft, ic, tag):
                ps = psm.tile([128, 512], F32, tag=tag, bufs=TB[tag], name="psqk")
                for ct in range(8):
                    nc.tensor.matmul(
                        ps,
                        wqkv_sb[:, ct, ft * 128:(ft + 1) * 128],
                        xT_sb[:, ct, ic * 512:(ic + 1) * 512],
                        start=(ct == 0), stop=(ct == 7),
                    )
                nc.vector.tensor_copy(qk_sb[:, ft, ic * 512:(ic + 1) * 512], ps)

            def v_group(nt, tag):
                ps = psm.tile([128, NH * HD], F32, tag=tag, bufs=TB[tag], name="psvp")
                for ct in range(8):
                    nc.tensor.matmul(
                        ps,
                        xT_sb[:, ct, nt * 128:(nt + 1) * 128],
                        wqkv_sb[:, ct, 512:768],
                        start=(ct == 0), stop=(ct == 7),
                    )
                for h in range(NH):
                    nc.vector.tensor_copy(
                        v_aug[:, nt, h, 0:HD], ps[:, h * HD:(h + 1) * HD]
                    )

            def y_group(it, fc, tag):
                psy = psm.tile([128, 512], F32, tag=tag, bufs=TB[tag], name="pyt")
                for pp in range(2):
                    nc.tensor.matmul(
                        psy,
                        o_sb[:, pp, it * 128:(it + 1) * 128],
                        wout_sb[:, pp, fc * 512:(fc + 1) * 512],
                        start=(pp == 0), stop=(pp == 1),
                    )
                y_sb = yb.tile([128, 512], F32, tag="ysb", name="ysbt")
                nc.vector.tensor_copy(y_sb, psy)
                nc.sync.dma_start(
                    y[it * 128:(it + 1) * 128, fc * 512:(fc + 1) * 512], y_sb)

            # pair-0 inputs (q01=ft0, k01=ft2) and V first so attention starts early
            for ic in range(4):
                qk_group(0, ic, "qk")
                qk_group(2, ic, "qk")
                for nt in range(4 * ic, 4 * ic + 4):
                    v_group(nt, "vp")
            for ic in range(4):
                qk_group(1, ic, "qk")
                qk_group(3, ic, "qk")

            # ---------------- attention + interleaved out-projection --------
            for p in range(2):  # head pair (heads 2p, 2p+1)
                for ic in range(4):  # query chunk (512)
                    i0 = ic * 512
                    po = [psm.tile([HD + 1, 512], F32, tag=f"o{e}", name=f"po{e}")
                          for e in range(2)]
                    for jt in range(16):  # key tile (128)
                        ps = psm.tile([128, 1024], F32, tag="sb", bufs=2,
                                      name="pss")
                        for e in range(2):  # row-group packed pair
                            pb = 64 * e
                            nc.tensor.matmul(
                                ps[:, e * 512:(e + 1) * 512],
                                qk_sb[pb:pb + 64, 2 + p, jt * 128:(jt + 1) * 128],
                                qk_sb[pb:pb + 64, p, i0:i0 + 512],
                                start=True, stop=True,
                                tile_position=(pb, 0),
                            )
                        pt = pP.tile([128, 1024], BF16, tag="p")
                        nc.scalar.activation(pt, ps, EXP, scale=SCALE)
                        for e in range(2):
                            nc.tensor.matmul(
                                po[e][0:HD + 1, :],
                                v_aug[:, jt, 2 * p + e, :],
                                pt[:, e * 512:(e + 1) * 512],
                                start=(jt == 0), stop=(jt == 15),
                            )
                    # normalize: copy out of PSUM, reciprocal of sums, broadcast
                    for e in range(2):
                        o_u = oup.tile([HD + 1, 512], F32, tag=f"ou{e}",
                                       name=f"ou{e}")
                        nc.vector.tensor_copy(o_u, po[e][0:HD + 1, :])
                        r0 = stat.tile([1, 512], F32, tag=f"r0{e}", name=f"r0{e}")
                        nc.sync.dma_start(r0, o_u[HD:HD + 1, :])
                        r1 = stat.tile([1, 512], F32, tag=f"r1{e}", name=f"r1{e}")
                        rs = stat.tile([1, 512], F32, tag=f"rs{e}", name=f"rs{e}")
                        nc.vector.reciprocal_approx_accurate(r1, r0, rs)
                        rb = rbc.tile([64, 512], F32, tag="rb")
                        nc.gpsimd.partition_broadcast(rb, r1)
                        if e == 0:
                            nc.vector.tensor_mul(
                                o_sb[0:64, p, i0:i0 + 512], o_u[0:64, :], rb
                            )
                        else:
                            tmp = shf.tile([64, 512], BF16, tag="tmp")
                            nc.vector.tensor_mul(tmp, o_u[0:64, :], rb)
                            nc.sync.dma_start(o_sb[64:128, p, i0:i0 + 512], tmp)
                    if p == 1:
                        for k in range(8):
                            y_group(4 * ic + k // 2, k % 2,
                                    "vp" if k % 2 else "qk")


_NC = None


def _get_nc():
    global _NC
    if _NC is None:
        _NC = build_program()
    return _NC


def make_in_maps(x, w_qkv, w_out):
    x = np.asarray(x, dtype=np.float32)
    w_qkv = np.asarray(w_qkv, dtype=np.float32)
    w_out = np.asarray(w_out, dtype=np.float32)
    xT = [np.ascontiguousarray(x[b].T) for b in range(B)]
    in_maps = []
    for c in range(NCORES):
        b, g = divmod(c, 4)
        f0 = g * NH * HD  # first feature col of this head group (256 wide)
        wq = w_qkv[:, f0:f0 + 256]
        wk = w_qkv[:, CDIM + f0:CDIM + f0 + 256]
        wv = w_qkv[:, 2 * CDIM + f0:2 * CDIM + f0 + 256]
        in_maps.append({
            "xT": xT[b],
            "wqkv": np.ascontiguousarray(np.concatenate([wq, wk, wv], axis=1)),
            "wout": np.ascontiguousarray(w_out[f0:f0 + 256, :]),
        })
    return in_maps


def kernel(x, w_qkv, b_qkv, w_out, b_out, _trace=False):
    """Full inputs in, full (B, N, C) output out. b_qkv is all-zeros by the
    problem's input spec (fill: zeros); b_out is added on the host."""
    nc = _get_nc()
    in_maps = make_in_maps(x, w_qkv, w_out)
    res = run_bass_kernel_spmd(nc, in_maps, core_ids=list(range(NCORES)),
                               trace=_trace)
    out = np.zeros((B, NSEQ, CDIM), dtype=np.float32)
    for c in range(NCORES):
        out[c // 4] += res.results[c]["y"]
    out += np.asarray(b_out, dtype=np.float32)
    if _trace:
        kernel.last_exec_time_ns = res.exec_time_ns
        kernel.last_results = res
    return out

